# revision 46
# baseline (speedup 1.0000x reference)
"""Trainium2 Bass kernel for nn_BoxLoss (YOLO-style box regression loss).

Contract: kernel(**inputs) takes FULL unsharded inputs (numpy), returns the
FULL scalar loss. Internally: pure data parallel over batch across 8
NeuronCores (4 images per core); each core computes its 12 (scale, image)
row losses entirely on-device and writes its partial sum; the host adds
the 8 partials while unsharding.

Only ~50 targets x 12 rows of real work exist per core; the big
[B,A,g,g,85] activation tensors are touched ONLY via indirect (gather)
DMAs of the <=600 matched cells x 4 channels the loss actually reads -
the kernel never streams the full tensors.

Single layout end-to-end: partition p = bh*50 + j (image-half, target),
free dim sbl = s*2 + bl (scale, image-parity), channels innermost. The
indirect-DMA HW consumes ONE index per destination partition, so the 600
(target, row) cells need exactly 6 gathers, fed directly by the computed
index tile - no cross-partition relayouts anywhere on the critical path.
The last-wins dedup compares int16 cell keys against a DRAM-broadcast
copy; the final reduction is partition-local via block-indicator PE
matmuls.
"""

import numpy as np

import concourse.bass as bass
import concourse.bacc as bacc
import concourse.mybir as mybir
import concourse.tile as tile
from concourse.tile import add_dep_helper

NCORES = 8
GRIDS = (52, 26, 13)
A = 3           # anchors per scale
T = 50          # targets per image
PB = 4          # images per core
SENT = 8112.0   # sentinel cell id for unmatched targets (>= any real cell)
B_TOTAL = 32
P100 = 2 * T    # partitions: (bh, j)
SBL = 6         # free rows: (s, bl)

F32 = mybir.dt.float32
I16 = mybir.dt.int16
I32 = mybir.dt.int32

_SCALE_ELEMS = [PB * A * g * g * 85 for g in GRIDS]
_SCALE_BASE = [0, _SCALE_ELEMS[0], _SCALE_ELEMS[0] + _SCALE_ELEMS[1]]
OUTCAT_ELEMS = sum(_SCALE_ELEMS)

# hostpack column layout ([100, _HP_TOT]) - runtime data + consts that vary
# only over (partition, sbl) grid structure
_H_TGT = 0        # [0,8)    raw targets (bl, c)
_H_AWH = 8        # [8,44)   anchor w/h (q, sbl, a)
_H_G24 = 44       # [44,68)  g per (sbl, c)
_H_BG = 68        # [68,74)  scale base + b*3*g^2*85  (b = 2bh+bl)
_H_HW = 74        # [74,80)  g^2
_H_W = 80         # [80,86)  g
_HP_TOT = 86

# cstI column layout ([100, 103]): identity-100 + onesU + one
_I_EYE = 0        # [0,100)
_I_ONESU = 100    # [100,102)
_I_ONE = 102      # [102,103)
_I_TOT = 103


def _host_consts():
    sbl = np.arange(SBL)
    s = sbl // 2
    g = np.array(GRIDS, dtype=np.float64)[s]              # [6]

    g24 = np.broadcast_to(g[:, None], (SBL, 4)).reshape(-1)       # [24]
    hw6 = g * g
    w6 = g
    p = np.arange(P100)
    bh = p // T
    base = np.array(_SCALE_BASE, dtype=np.float64)[s][None, :]
    b = (2 * bh[:, None] + (sbl % 2)[None, :])
    bg = base + b * (A * 85) * (g ** 2)[None, :]          # [100, 6]

    row = np.concatenate([
        np.zeros(8), np.zeros(36), g24, np.zeros(6), hw6, w6])
    hp_const = np.broadcast_to(row, (P100, _HP_TOT)).copy()
    hp_const[:, _H_BG:_H_BG + 6] = bg
    return hp_const.astype(np.float32)


def _inline_consts():
    # LATER2 [100, 300] int16: col = sbl*50 + k -> 1 if k > j (= p % 50)
    j = (np.arange(P100) % T)[:, None]
    k = np.tile(np.arange(T), SBL)[None, :]
    lat2 = (k > j).astype(np.int16)

    cstI = np.zeros((P100, _I_TOT), np.float32)
    cstI[:, _I_EYE:_I_EYE + P100] = np.eye(P100, dtype=np.float32)
    cstI[0:T, _I_ONESU] = 1.0
    cstI[T:P100, _I_ONESU + 1] = 1.0
    cstI[:, _I_ONE] = 1.0
    return np.ascontiguousarray(lat2), np.ascontiguousarray(cstI)


def build_nc(use_collective: bool = False):
    nc = bacc.Bacc("TRN2", target_bir_lowering=False, debug=False,
                   num_devices=NCORES)

    hp_d = nc.dram_tensor("hostpack", [P100, _HP_TOT], F32, kind="ExternalInput")
    outcat_d = nc.dram_tensor("outcat", [OUTCAT_ELEMS], F32, kind="ExternalInput")
    loss_d = nc.dram_tensor("loss", [1, 1], F32, kind="ExternalOutput")
    lat2_np, cstI_np = _inline_consts()
    lat2_d = nc.inline_tensor(lat2_np, name="lat2")
    cstI_d = nc.inline_tensor(cstI_np, name="cstI")

    AL = mybir.AluOpType
    AX = mybir.AxisListType.X

    with tile.TileContext(nc) as tc:
        with (
            tc.tile_pool(name="sbuf", bufs=1) as sp,
            tc.tile_pool(name="psum", bufs=1, space="PSUM") as pp,
            tc.tile_pool(name="dram", bufs=1, space="DRAM") as dp,
        ):
            def tt(out, in0, in1, op):
                return nc.vector.tensor_tensor(out=out, in0=in0, in1=in1, op=op)

            def ts(out, in0, s1, op, s2=None, op2=None):
                if op2 is None:
                    return nc.vector.tensor_scalar(out=out, in0=in0, scalar1=s1,
                                                   scalar2=None, op0=op)
                return nc.vector.tensor_scalar(out=out, in0=in0, scalar1=s1,
                                               scalar2=s2, op0=op, op1=op2)

            def stt(out, in0, scalar, in1, op0, op1):
                return nc.vector.scalar_tensor_tensor(
                    out=out, in0=in0, scalar=scalar, in1=in1, op0=op0, op1=op1)

            _tn = [0]

            def new(shape, dt=F32):
                _tn[0] += 1
                return sp.tile(shape, dt, name=f"t{_tn[0]}")

            # ---------- loads (readiness-ordered per ring) ----------
            # sync ring: hostpack, kd, keyB-lo, out
            # ACT ring:  cstI, lat2, keyB-hi
            hp = new([P100, _HP_TOT])
            nc.sync.dma_start(out=hp[:], in_=hp_d[:, :])
            cstI = new([P100, _I_TOT])
            nc.scalar.dma_start(out=cstI[:], in_=cstI_d[:, :])
            lat = new([P100, SBL * T], I16)
            nc.scalar.dma_start(out=lat[:], in_=lat2_d[:, :])

            def C(c0, w):
                return hp[:, c0:c0 + w]

            tgt = C(_H_TGT, 8)
            awh2 = C(_H_AWH, 36)
            EYE = cstI[:, _I_EYE:_I_EYE + P100]
            onesU = cstI[:, _I_ONESU:_I_ONESU + 2]
            ones2 = cstI[0:2, _I_ONE:_I_ONE + 1]

            # ---------- validity ----------
            sv = new([P100, 2])
            nc.vector.reduce_sum(out=sv[:],
                                 in_=tgt.rearrange("p (bl c) -> p bl c", c=4),
                                 axis=AX)
            v2 = new([P100, 2]); ts(v2[:], sv[:], 0.0, AL.is_gt)

            # ---------- t = raw * g ----------
            t4 = new([P100, 24])
            tt(t4[:], tgt[:, None, :].to_broadcast([P100, 3, 8]), C(_H_G24, 24),
               AL.mult)
            t4v = t4[:].rearrange("p (sbl c) -> p sbl c", c=4)
            txy = t4v[:, :, 0:2]
            twh = t4v[:, :, 2:4]

            # ---------- floor(xy) ----------
            r2 = new([P100, 12])
            ts(r2[:], txy, float(2 ** 23), AL.add, -float(2 ** 23), AL.add)
            gtm = new([P100, 12])
            tt(gtm[:], r2[:], txy, AL.is_gt)
            fxy = new([P100, 12])
            tt(fxy[:], r2[:], gtm[:], AL.subtract)
            fv = fxy[:].rearrange("p (sbl q) -> p sbl q", q=2)
            cx = fv[:, :, 0:1]
            cy = fv[:, :, 1:2]

            # ---------- target rect ----------
            zt05 = new([P100, 12])
            stt(zt05[:], txy, -0.5, fxy[:], AL.add, AL.subtract)
            lo = new([P100, 12])
            stt(lo[:], twh, -0.5, zt05[:], AL.mult, AL.add)
            hi = new([P100, 12])
            stt(hi[:], twh, 0.5, zt05[:], AL.mult, AL.add)

            # ---------- anchors + IoU in (q, sbl, a) layout ----------
            awhh = new([P100, 36]); ts(awhh[:], awh2, 0.5, AL.mult)
            nawhh = new([P100, 36]); ts(nawhh[:], awh2, -0.5, AL.mult)
            areaa = new([P100, 18])
            tt(areaa[:], awh2[:, 0:18], awh2[:, 18:36], AL.mult)

            def bcQ(t12):
                # [100,(sbl,q)] -> [100,(q,sbl,a)] = [100,36]
                return (t12[:].rearrange("p (sbl q) -> p q sbl", q=2)
                        [:, :, :, None].to_broadcast([P100, 2, SBL, 3]))

            P0 = new([P100, 36]); tt(P0[:], bcQ(lo), nawhh[:], AL.max)
            P1 = new([P100, 36]); tt(P1[:], bcQ(hi), awhh[:], AL.min)
            D = new([P100, 36]); tt(D[:], P1[:], P0[:], AL.subtract)
            M0 = new([P100, 36]); ts(M0[:], D[:], 0.0, AL.max)
            inter = new([P100, 18])
            tt(inter[:], M0[:, 0:18], M0[:, 18:36], AL.mult)
            dT = new([P100, 12]); tt(dT[:], hi[:], lo[:], AL.subtract)
            dv = dT[:].rearrange("p (sbl q) -> p sbl q", q=2)
            areat = new([P100, 6]); tt(areat[:], dv[:, :, 0:1], dv[:, :, 1:2], AL.mult)
            un1 = new([P100, 18])
            tt(un1[:], areat[:, :, None].to_broadcast([P100, SBL, 3]), areaa[:],
               AL.add)
            union = new([P100, 18]); tt(union[:], un1[:], inter[:], AL.subtract)
            runi = new([P100, 18]); nc.vector.reciprocal(out=runi[:], in_=union[:])
            iou = new([P100, 18]); tt(iou[:], inter[:], runi[:], AL.mult)

            # ---------- overlap / argmax / cell / gather offsets ----------
            overlap = new([P100, 6])
            nc.vector.reduce_max(out=overlap[:],
                                 in_=iou[:].rearrange("p (sbl a) -> p sbl a", a=3),
                                 axis=AX)
            iv = iou[:].rearrange("p (sbl a) -> p sbl a", a=3)
            eqB = new([P100, 12])
            tt(eqB[:], iv[:, :, 0:2],
               overlap[:, :, None].to_broadcast([P100, SBL, 2]), AL.is_equal)
            ev = eqB[:].rearrange("p (sbl e) -> p sbl e", e=2)
            t2 = new([P100, 6])
            ts(t2[:], ev[:, :, 1:2], 0.0, AL.is_equal, 1.0, AL.add)
            neq0 = new([P100, 6]); ts(neq0[:], ev[:, :, 0:1], 0.0, AL.is_equal)
            anc = new([P100, 6]); tt(anc[:], neq0[:], t2[:], AL.mult)

            ca = new([P100, 6]); tt(ca[:], anc[:], C(_H_HW, 6), AL.mult)
            cb = new([P100, 6]); tt(cb[:], cy, C(_H_W, 6), AL.mult)
            cc = new([P100, 6]); tt(cc[:], ca[:], cb[:], AL.add)
            cell = new([P100, 6]); tt(cell[:], cc[:], cx, AL.add)
            idxi = new([P100, 6], I32)
            stt(idxi[:], cell[:], 85.0, C(_H_BG, 6), AL.mult, AL.add)

            # ---------- dedup key (overlaps the gathers) ----------
            om = new([P100, 6]); ts(om[:], overlap[:], 0.5, AL.is_gt)
            m = new([P100, 6])
            tt(m[:], om[:].rearrange("p (s bl) -> p s bl", bl=2),
               v2[:, None, :].to_broadcast([P100, 3, 2]), AL.mult)
            kk = new([P100, 6])
            stt(kk[:], cell[:], -SENT, m[:], AL.add, AL.mult)
            key = new([P100, 6]); ts(key[:], kk[:], SENT, AL.add)
            key16 = new([P100, 6], I16)
            ts(key16[:], kk[:], SENT, AL.add)

            keyT_p = pp.tile([SBL, P100], F32, name="keyT_p")
            nc.tensor.matmul(out=keyT_p[:], lhsT=key[:], rhs=EYE,
                             start=True, stop=True)
            keyT = new([SBL, P100], I16)
            nc.vector.tensor_copy(out=keyT[:], in_=keyT_p[:])
            # DRAM roundtrip for the partition-broadcast, pipelined in
            # independent bh-halves across the two HWDGE rings.
            kd = nc.dram_tensor("kd", [SBL * P100], I16)
            kdw = kd[:].rearrange("(q bh k) -> q bh k", bh=2, k=T)
            nc.sync.dma_start(out=kdw[:, 0, :], in_=keyT[:, 0:T])
            nc.scalar.dma_start(out=kdw[:, 1, :], in_=keyT[:, T:P100])
            # keyB[p=(bh,j), sbl*50+k] = key[bh*50+k, sbl]
            keyB = new([P100, SBL * T], I16)
            nc.sync.dma_start(
                out=keyB[0:T, :],
                in_=kdw[:, 0, :].unsqueeze(0).to_broadcast([T, SBL, T]))
            nc.scalar.dma_start(
                out=keyB[T:P100, :],
                in_=kdw[:, 1, :].unsqueeze(0).to_broadcast([T, SBL, T]))

            # rsqrt of t_wh
            rwh2 = new([P100, 12])
            nc.vector.reciprocal(out=rwh2[:], in_=twh)
            rstw = new([P100, 12]); nc.scalar.sqrt(out=rstw[:], in_=rwh2[:])

            # ---------- 6 indirect gathers (3 pair tiles) ----------
            gpair = [new([P100, 8]) for _ in range(3)]
            for q in (0, 2, 4, 1, 3, 5):   # adjacent gathers hit distinct tiles
                s_, bl = q // 2, q % 2
                nc.gpsimd.indirect_dma_start(
                    out=gpair[s_][:, bl * 4:(bl + 1) * 4], out_offset=None,
                    in_=outcat_d[:].unsqueeze(1),
                    in_offset=bass.IndirectOffsetOnAxis(ap=idxi[:, q:q + 1],
                                                        axis=0),
                )

            TS2 = new([P100, 6])
            winner2 = new([P100, 12])  # cols 0:6 winner, cols 6:12 winner*TS

            # dedup tail (keyB lands before gather pair 0 completes)
            E = new([P100, SBL * T], I16)
            tt(E[:], key16[:, :, None].to_broadcast([P100, SBL, T]), keyB[:],
               AL.is_equal)
            EL = new([P100, SBL * T], I16)
            tt(EL[:], E[:], lat[:], AL.mult)
            ov = new([P100, 6], I16)
            nc.vector.reduce_max(out=ov[:],
                                 in_=EL[:].rearrange("p (sbl k) -> p sbl k", k=T),
                                 axis=AX)
            nov = new([P100, 6]); ts(nov[:], ov[:], 0.0, AL.is_equal)
            last_dedup = tt(winner2[:, 0:6], m[:], nov[:], AL.mult)

            def stripe_chain(s_, order_after=None):
                g8 = gpair[s_]
                gv = g8[:].rearrange("p (bl c) -> p bl c", c=4)
                t8 = t4v[:, 2 * s_:2 * s_ + 2, :]
                rcpw = new([P100, 4])
                ri = nc.vector.reciprocal(out=rcpw[:], in_=gv[:, :, 2:4])
                if order_after is not None:
                    # ri waits on order_after: dedup tail fills the gather
                    # window before this stripe's chain runs
                    add_dep_helper(ri.ins, order_after.ins, True,
                                   "keep dedup ahead of last stripe chain")
                rspw = new([P100, 4]); nc.scalar.sqrt(out=rspw[:], in_=rcpw[:])
                sel = new([P100, 8])
                selv = sel[:].rearrange("p (bl c) -> p bl c", c=4)
                sx = tt(selv[:, :, 0:2], gv[:, :, 0:2], t8[:, :, 0:2],
                        AL.subtract)
                if order_after is not None:
                    add_dep_helper(sx.ins, order_after.ins, True,
                                   "keep dedup ahead of last stripe chain")
                tt(selv[:, :, 2:4], rspw[:], rstw[:, 4 * s_:4 * s_ + 4],
                   AL.subtract)
                sq = new([P100, 8]); tt(sq[:], sel[:], sel[:], AL.mult)
                nc.vector.reduce_sum(
                    out=TS2[:, 2 * s_:2 * s_ + 2],
                    in_=sq[:].rearrange("p (bl c) -> p bl c", c=4), axis=AX)

            stripe_chain(0)
            stripe_chain(1)
            stripe_chain(2, order_after=last_dedup)

            # ---------- partition-local final reduction ----------
            tt(winner2[:, 6:12], TS2[:], winner2[:, 0:6], AL.mult)
            M1_p = pp.tile([2, 12], F32, name="M1_p")
            nc.tensor.matmul(out=M1_p[:], lhsT=onesU, rhs=winner2[:],
                             start=True, stop=True)
            mx2 = new([2, 6])
            ts(mx2[:], M1_p[:, 0:6], 1.0, AL.max, 2.0, AL.mult)
            rden2 = new([2, 6]); nc.vector.reciprocal(out=rden2[:], in_=mx2[:])
            rl2 = new([2, 6]); tt(rl2[:], M1_p[:, 6:12], rden2[:], AL.mult)
            pt2 = new([2, 1])
            nc.vector.reduce_sum(out=pt2[:], in_=rl2[:], axis=AX)
            tot_p = pp.tile([1, 1], F32, name="tot_p")
            nc.tensor.matmul(out=tot_p[:], lhsT=ones2, rhs=pt2[:],
                             start=True, stop=True)
            p32 = new([1, 1])
            ts(p32[:], tot_p[:], 1.0 / B_TOTAL, AL.mult)

            if use_collective:
                ccin = dp.tile([1, 1], F32, name="ccin")
                ccout = dp.tile([1, 1], F32, name="ccout")
                nc.sync.dma_start(out=ccin[:], in_=p32[:])
                nc.gpsimd.collective_compute(
                    "AllReduce", AL.add,
                    replica_groups=[list(range(NCORES))],
                    ins=[ccin[:].opt()], outs=[ccout[:].opt()],
                )
                nc.sync.dma_start(out=loss_d[:, :], in_=ccout[:])
            else:
                nc.sync.dma_start(out=loss_d[:, :], in_=p32[:])

    nc.compile()
    return nc


_HOST_CONSTS = _host_consts()


def make_in_maps(output0, anchors0, output1, anchors1, output2, anchors2,
                 targets):
    outs = [np.asarray(output0), np.asarray(output1), np.asarray(output2)]
    ancs = [np.asarray(anchors0), np.asarray(anchors1), np.asarray(anchors2)]
    tg = np.asarray(targets)

    # anchor block (q, sbl, a): col = q*18 + (s*2+bl)*3 + a
    awh_row = np.zeros(36, np.float32)
    for q_, col in ((0, 0), (1, 1)):
        for s_ in range(3):
            for bl in range(2):
                for a_ in range(3):
                    awh_row[q_ * 18 + (s_ * 2 + bl) * 3 + a_] = ancs[s_][a_, col]

    in_maps = []
    for c in range(NCORES):
        sl = slice(c * PB, (c + 1) * PB)
        raw = tg[sl, :, 1:5].astype(np.float32)          # [4, 50, 4]
        tg8 = (raw.reshape(2, 2, T, 4)                    # (bh, bl, j, c)
               .transpose(0, 2, 1, 3).reshape(P100, 8))   # (bh,j) x (bl,c)
        hostpack = _HOST_CONSTS.copy()
        hostpack[:, _H_TGT:_H_TGT + 8] = tg8
        hostpack[:, _H_AWH:_H_AWH + 36] = awh_row[None, :]
        outcat = np.concatenate([o[sl].ravel() for o in outs]).astype(np.float32)
        in_maps.append({"hostpack": np.ascontiguousarray(hostpack),
                        "outcat": outcat})
    return in_maps


_NC_CACHE = {}


def kernel(output0, anchors0, output1, anchors1, output2, anchors2, targets):
    import time
    from concourse.bass_utils import run_bass_kernel_spmd

    if "nc" not in _NC_CACHE:
        _NC_CACHE["nc"] = build_nc(use_collective=False)
    nc = _NC_CACHE["nc"]
    in_maps = make_in_maps(output0, anchors0, output1, anchors1, output2,
                           anchors2, targets)
    res = None
    for attempt in range(3):
        try:
            res = run_bass_kernel_spmd(nc, in_maps, list(range(NCORES)))
            break
        except Exception:
            # transient NRT device errors have been observed; back off + retry
            if attempt == 2:
                raise
            time.sleep(20.0 * (attempt + 1))
    total = np.float32(0.0)
    for c in range(NCORES):
        total += np.float32(res.results[c]["loss"].reshape(()))
    return np.float32(total)
